# revision 21
# baseline (speedup 1.0000x reference)
"""CIEDE2000 ColorLoss kernel for Trainium2, 8 NeuronCores, data-parallel.

Full inputs x, y: [32, 3, 512, 512] f32 NCHW in [0, 1].
Output: scalar f32 = mean(ciede2000(rgb2lab(x), rgb2lab(y))) / 100.

Sharding: batch dim split 4 images per core (8 cores). Each core computes a
per-partition sum of deltaE over its 4*512*512 pixels; host combines.

Design (v2):
  - Scalar (ACT) engine does ONLY Ln/Exp -> single table set, zero
    ACT_TABLE_LOAD switches after warmup.
  - Zero GpSimd compute (it contends with the DVE on the shared SBUF port).
  - Hue handled algebraically: unit bisector (cos hbar, sin hbar) via
    vector addition u1*C2' + u2*C1'; T weighting as parity-split
    polynomial P(c) + s*Q(c); dtheta gaussian approximated as
    exp(10.5*(cos(hbar-275deg)-1)); dHp = 2000*cross*sqrt(C1'C2')/|v|
    (exact identity, no trig, sign included).
  - Fused custom DVE ops (selects, sum-of-squares, lincombs, polys) with
    immediate constants; bf16 planes for 2x stock-DVE throughput.
  - Math error vs reference ~5e-4 (numpy-simulated, bf16 rounding incl).
"""
import os
import sys

sys.path.insert(0, "/opt/trn_rl_repo")

import numpy as np
import concourse.bacc as bacc
import concourse.tile as tile
import concourse.mybir as mybir
import concourse.dve_ops as D
from concourse.dve_spec import (
    Spec, Src0, Src1, C0, C1, C2, relu, sq, select, maxx, minn,
    lower as dve_lower, _has_src1,
)
from concourse.dve_uop import DveOpSpec
from concourse.bass_utils import run_bass_kernel_spmd
from contextlib import ExitStack

F32 = mybir.dt.float32
BF16 = mybir.dt.bfloat16
AF = mybir.ActivationFunctionType
ALU = mybir.AluOpType

P = 128          # partitions
FCH = 1024       # chunk free dim
NCHUNK = 8       # chunks per core: P*FCH*NCHUNK = 1048576 px = 4 imgs
NCORE = 8
IMGS_PER_CORE = 4
ROWS_PER_IMG = 32  # partitions per image: 262144 / 8192

# ---- constants -------------------------------------------------------------
PI = float(np.pi)
LNP = float(np.log(1.0 / 128.0))     # ln((25/50)^7)
B_LIN = float(0.055 / 1.055)
K1 = float(PI / 3)
K3 = float(-(PI / 3) ** 3 / 6.0)
K5 = float((PI / 3) ** 5 / 120.0)

_M = np.array([[0.412453, 0.357580, 0.180423],
               [0.212671, 0.715160, 0.072169],
               [0.019334, 0.119193, 0.950227]], dtype=np.float64)
_W = np.array([0.95047, 1.0, 1.08883], dtype=np.float64)
MW = (_M / _W[:, None]).astype(np.float32)  # [3,3]

_D2R = PI / 180.0


def _cd(d):
    return float(np.cos(d * _D2R))


def _sd(d):
    return float(np.sin(d * _D2R))


# T(h) = P(cos h) + sin(h) * Q(cos h), parity-split Chebyshev coefficients
TA0 = 1.0 - 0.24 - 0.20 * _cd(63)
TA1 = -0.17 * _cd(30) - 0.96 * _cd(6)
TA2 = 0.48 + 1.6 * _cd(63)
TA3 = 1.28 * _cd(6)
TA4 = -1.6 * _cd(63)
TQ0 = -0.17 * _sd(30) + 0.32 * _sd(6)
TQ1 = 0.80 * _sd(63)
TQ2 = -1.28 * _sd(6)
TQ3 = -1.60 * _sd(63)
C275 = _cd(275)
S275 = _sd(275)

_BIASES = [0.0, B_LIN, LNP, 1.0, 20.0, 1e-12, -10.5, 1e-30]

_NC_CACHE = {}


# ---- custom DVE ops --------------------------------------------------------
def _register_op(name, spec, subdim=False):
    if name in D._SUB_OPCODE_FOR_NAME:
        return next(o for o in D.OPS if o.name == name)
    row = 1 + len(D.OPS)
    assert row < 0x20, "custom DVE opcode rows exhausted"
    D._SUB_OPCODE_FOR_NAME[name] = row
    shas = {}
    for ver in ("v3",):
        s = DveOpSpec(name=name, opcode=row, uops=dve_lower(spec, ver=ver),
                      rd1_en=_has_src1(spec))
        shas[ver] = s.sha(ver)
    op = D.DveOp(name, spec, subdim, shas)
    D.OPS.append(op)
    D.CUSTOM_DVE_SPECS[name] = spec
    return op


# select(x > c0, a, x*c1 + c2) : sRGB + cbrt branch
OP_SEL_GT = _register_op("ANT_SEL_GT", Spec(
    body=select(Src0 > C0, Src1, Src0 * C1 + C2)))
# sq(a*c0) + sq(b*c1) : chroma^2, |v|^2, dE^2 partials
OP_SUMSQ = _register_op("ANT_SUMSQ", Spec(
    body=sq(Src0 * C0) + sq(Src1 * C1)))
# a*c0 + b*c1 + c2
OP_LINCOMB = _register_op("ANT_LINCOMB", Spec(
    body=Src0 * C0 + Src1 * C1 + C2))
# sq((a+b)*c0 + c1) : L50^2
OP_ADD_AFF_SQ = _register_op("ANT_ADD_AFF_SQ", Spec(
    body=sq((Src0 + Src1) * C0 + C1)))
# (a*c0)*b + c1 : SL, SH
OP_MULMUL_ADD = _register_op("ANT_MULMUL_ADD", Spec(
    body=(Src0 * C0) * Src1 + C1))
# sq(a*c0)*sq(b) : zL
OP_SQSQ_MUL = _register_op("ANT_SQSQ_MUL", Spec(
    body=sq(Src0 * C0) * sq(Src1)))
# (a*c0 + c1)*b : (1+G)*alpha, T parity pieces
OP_AFF_MUL = _register_op("ANT_AFF_MUL", Spec(
    body=(Src0 * C0 + C1) * Src1))
# sq(a)*c0 + a*c1 + c2 : quad poly
OP_QUAD = _register_op("ANT_QUAD", Spec(
    body=sq(Src0) * C0 + Src0 * C1 + C2))
# (sq(a)*b + c0)*a : sin odd-poly tail
OP_SIN_POLY = _register_op("ANT_SIN_POLY", Spec(
    body=(sq(Src0) * Src1 + C0) * Src0))
# relu(a + b) : final F clamp
OP_ADD_RELU = _register_op("ANT_ADD_RELU", Spec(
    body=relu(Src0 + Src1)))
# clamp(a*c0*b, c1, c2) : unit-vector components (degenerate-hue guard)
OP_SMUL_CLAMP = _register_op("ANT_SMUL_CLAMP", Spec(
    body=minn(maxx(Src0 * C0 * Src1, C1), C2)))
# clamp(a, c1, c2)*b*c0 : dHp = 200*g12*clamp(sin(dh/2))
OP_CLAMP_MUL = _register_op("ANT_CLAMP_MUL", Spec(
    body=minn(maxx(Src0, C1), C2) * Src1 * C0))
# select(a <= c0, b, a*c1 + c2) : cbrt branch keyed on cbr itself
OP_SEL_LE = _register_op("ANT_SEL_LE", Spec(
    body=select(Src0 <= C0, Src1, Src0 * C1 + C2)))


# Force Ln and Exp to resolve to the combined natural_log_exp set: the
# greedy table-load pass otherwise alternates natural_log <-> exp_and_others
# on every Ln/Exp switch (~2.7us per ACT_TABLE_LOAD, ~20 per chunk).
_ORIG_GAT = None


def _install_lnexp_table_patch():
    global _ORIG_GAT
    if _ORIG_GAT is not None:
        return
    import concourse.hw_specs as hw_specs
    _ORIG_GAT = hw_specs.get_activation_tables

    def _gat(arch):
        t = _ORIG_GAT(arch)
        out = {}
        for name, fns in t.items():
            if name != "natural_log_exp_and_others":
                fns = {f for f in fns if f not in (AF.Ln, AF.Exp)}
            out[name] = fns
        return out

    hw_specs.get_activation_tables = _gat
    bacc.get_activation_tables = _gat


def _reg_consts(nc, values):
    for v in values:
        v = float(v)
        if (F32, v) not in nc.const_aps.aps:
            t = nc.alloc_sbuf_tensor(f"constf32_{repr(v)}", [128, 1], F32)
            nc.gpsimd.memset(t.ap(), v)
            nc.const_aps.aps[(F32, v)] = t.ap()
    nc.all_engine_barrier()


def build_nc(debug_dump=False):
    _install_lnexp_table_patch()
    nc = bacc.Bacc("TRN2", target_bir_lowering=False, debug=False)
    _reg_consts(nc, _BIASES)
    A = nc.scalar
    V = nc.vector

    # inputs viewed as [img, ch, row, chunk, col]
    shp = [IMGS_PER_CORE, 3, ROWS_PER_IMG, NCHUNK, FCH]
    x_d = nc.dram_tensor("x", shp, F32, kind="ExternalInput").ap()
    y_d = nc.dram_tensor("y", shp, F32, kind="ExternalInput").ap()
    out_d = nc.dram_tensor("out", [P, 1], F32, kind="ExternalOutput").ap()

    dbg_tiles = {}

    def dbg(name, t):
        if debug_dump:
            dbg_tiles[name] = (t, t.shape[1], t.dtype)

    W = FCH          # single-plane width
    W2 = 2 * FCH     # pair width
    W6 = 6 * FCH

    with tile.TileContext(nc) as tc, ExitStack() as ctx:
        inpool = ctx.enter_context(tc.tile_pool(name="in", bufs=1))
        pool = ctx.enter_context(tc.tile_pool(name="main", bufs=1))

        NTMP = 8
        NTMPF = 5
        tmp_i = [0, 0]

        def T_(tag, w=W, dt=BF16):
            return pool.tile([P, w], dt, tag=tag, name=tag)

        def tmp(dt=BF16):
            if dt is F32:
                tag = f"ftmp{tmp_i[1] % NTMPF}"
                tmp_i[1] += 1
            else:
                tag = f"tmp{tmp_i[0] % NTMP}"
                tmp_i[0] += 1
            return pool.tile([P, W], dt, tag=tag, name=tag)

        acc = pool.tile([P, NCHUNK], F32, tag="acc", name="acc")

        def S(t, i, n=1):
            """Free-dim slice covering planes [i, i+n) of a multi-plane tile."""
            return t[:, i * FCH:(i + n) * FCH]

        def stage_A(k):
            """DMA chunk k + srgb gamma (Act-heavy)."""
            IN = inpool.tile([P, W6], F32, tag="in6", name="in6")
            for c in range(3):
                for img, src in ((0, x_d), (1, y_d)):
                    pl = 2 * c + img
                    for im in range(IMGS_PER_CORE):
                        nc.sync.dma_start(
                            IN[im * ROWS_PER_IMG:(im + 1) * ROWS_PER_IMG,
                               pl * FCH:(pl + 1) * FCH],
                            src[im, c, :, k, :],
                        )
            W1 = T_("w1", W6)   # LNU -> GAM -> LIN
            A.activation(W1[:], IN[:], AF.Ln, scale=float(1 / 1.055),
                         bias=B_LIN)
            A.activation(W1[:], W1[:], AF.Exp, scale=2.4)
            INB = T_("inb", W6)
            A.activation(INB[:], IN[:], AF.Copy, scale=float(1 / 12.92))
            return INB, W1

        def stage_B(st):
            """lin max-blend + xyz (V-heavy)."""
            INB, W1 = st
            V.tensor_tensor(W1[:], INB[:], W1[:], ALU.max)
            XYZ = T_("xyz", W6)
            for kk in range(3):
                mk = MW[kk]
                t0 = pool.tile([P, W2], BF16, tag="xta", name="xta")
                t1 = pool.tile([P, W2], BF16, tag="xtb", name="xtb")
                V.tensor_scalar(t0[:], S(W1, 0, 2), float(mk[0]), None,
                                ALU.mult)
                V.tensor_scalar(t1[:], S(W1, 2, 2), float(mk[1]), None,
                                ALU.mult)
                V.tensor_add(t0[:], t0[:], t1[:])
                V.tensor_scalar(t1[:], S(W1, 4, 2), float(mk[2]), None,
                                ALU.mult)
                V.tensor_add(S(XYZ, 2 * kk, 2), t0[:], t1[:])
            return (XYZ,)

        def stage_C(st):
            """cbrt Ln/Exp + line plane (Act-heavy)."""
            (XYZ,) = st
            LINE = T_("line", W6)
            A.activation(LINE[:], XYZ[:], AF.Copy, scale=7.787,
                         bias=0.13793103)
            V.tensor_scalar(LINE[:], LINE[:], 0.20689655, None, ALU.min)
            WF = T_("wf", W6)   # LNX -> CBR -> F
            A.activation(WF[:], XYZ[:], AF.Ln)
            A.activation(WF[:], WF[:], AF.Exp, scale=float(1 / 3))
            return WF, LINE

        def stage_D(st, k):
            """cbrt select + full back-end for chunk k."""
            WF, LINE = st
            # f = max(cbr, min(line, cbrt(eps))) -- exact branch blend
            V.tensor_tensor(WF[:], WF[:], LINE[:], ALU.max)

            AL = T_("al", W2)
            BE = T_("be", W2)
            V.tensor_sub(AL[:], S(WF, 0, 2), S(WF, 2, 2))
            V.tensor_sub(BE[:], S(WF, 2, 2), S(WF, 4, 2))

            # ---- L chain ----
            fysum = tmp()
            V.tensor_add(fysum[:], S(WF, 2), S(WF, 3))
            L50 = T_("l50")
            A.activation(L50[:], fysum[:], AF.Square, scale=58.0, bias=-66.0)
            lld = tmp()
            A.activation(lld[:], L50[:], AF.Ln, bias=20.0)
            rLd = tmp()
            A.activation(rLd[:], lld[:], AF.Exp, scale=-0.5)
            SLf = T_("slf", W, F32)
            V._custom_dve(OP_MULMUL_ADD, out=SLf[:], in0=L50[:], in1=rLd[:],
                          s0=0.015, s1=1.0)
            rL = T_("rl", W, F32)
            V.reciprocal_approx_fast(rL[:], SLf[:])
            dfy = T_("dfy")
            V.tensor_sub(dfy[:], S(WF, 3), S(WF, 2))
            zL = T_("zl")
            V._custom_dve(OP_SQSQ_MUL, out=zL[:], in0=dfy[:], in1=rL[:],
                          s0=116.0)

            # ---- chroma + G ----
            C2p = T_("c2p", W2)
            V._custom_dve(OP_SUMSQ, out=C2p[:], in0=AL[:], in1=BE[:],
                          s0=5.0, s1=2.0)
            A.activation(C2p[:], C2p[:], AF.Ln)
            CCp = C2p
            A.activation(CCp[:], CCp[:], AF.Exp, scale=0.5)
            Sc = tmp()
            V.tensor_add(Sc[:], S(CCp, 0), S(CCp, 1))
            lcS = tmp()
            A.activation(lcS[:], Sc[:], AF.Ln)
            e1 = tmp()
            A.activation(e1[:], lcS[:], AF.Exp, scale=-7.0, bias=LNP)
            l1g = tmp()
            A.activation(l1g[:], e1[:], AF.Ln, bias=1.0)
            rsqG = tmp()
            A.activation(rsqG[:], l1g[:], AF.Exp, scale=-0.5)
            g1 = tmp()
            A.activation(g1[:], rsqG[:], AF.Copy, scale=-0.5, bias=1.5)
            AP = T_("ap", W2)
            V.tensor_mul(S(AP, 0), g1[:], S(AL, 0))
            V.tensor_mul(S(AP, 1), g1[:], S(AL, 1))

            CP2 = T_("cp2", W2)
            V._custom_dve(OP_SUMSQ, out=CP2[:], in0=AP[:], in1=BE[:],
                          s0=5.0, s1=2.0)
            LP = CP2
            A.activation(LP[:], CP2[:], AF.Ln)
            # CPS = [Cpy | Cpx]  (swapped -> pair products via one TT mult)
            CPS = T_("cps", W2)
            A.activation(S(CPS, 0), S(LP, 1), AF.Exp, scale=0.5)
            A.activation(S(CPS, 1), S(LP, 0), AF.Exp, scale=0.5)
            dCp = T_("dcp")
            V.tensor_sub(dCp[:], S(CPS, 0), S(CPS, 1))
            Scp = T_("scp")
            V.tensor_add(Scp[:], S(CPS, 0), S(CPS, 1))
            # SCH = [SC | SH] fp32 for the paired reciprocal
            SCH = T_("sch", W2, F32)
            A.activation(S(SCH, 0), Scp[:], AF.Copy, scale=2.25, bias=1.0)
            lcp = tmp()
            A.activation(lcp[:], Scp[:], AF.Ln)
            e2 = tmp()
            A.activation(e2[:], lcp[:], AF.Exp, scale=-7.0, bias=LNP)
            l2g = tmp()
            A.activation(l2g[:], e2[:], AF.Ln, bias=1.0)
            rsqC = T_("rsqc")
            A.activation(rsqC[:], l2g[:], AF.Exp, scale=-0.5)

            # ---- hue: cross, sqrt(C1C2), bisector (cb, sb) ----
            # fp32 chain: bf16*bf16 products are exact in fp32, keeping the
            # dHp identity exact; clamps guard the near-antipodal tail.
            m1 = tmp(F32)
            V.tensor_mul(m1[:], S(AP, 0), S(BE, 1))
            m2 = tmp(F32)
            V.tensor_mul(m2[:], S(AP, 1), S(BE, 0))
            cross = T_("cross", W, F32)
            V.tensor_sub(cross[:], m1[:], m2[:])
            lsum = tmp()
            V.tensor_add(lsum[:], S(LP, 0), S(LP, 1))
            g12 = tmp()
            A.activation(g12[:], lsum[:], AF.Exp, scale=0.25)
            prA = pool.tile([P, W2], BF16, tag="pra", name="pra")
            V.tensor_mul(prA[:], AP[:], CPS[:])
            vx = tmp(F32)
            V.tensor_add(vx[:], S(prA, 0), S(prA, 1))
            prB = pool.tile([P, W2], BF16, tag="pra", name="prb")
            V.tensor_mul(prB[:], BE[:], CPS[:])
            vy = tmp(F32)
            V.tensor_add(vy[:], S(prB, 0), S(prB, 1))
            n2 = tmp(F32)
            V._custom_dve(OP_SUMSQ, out=n2[:], in0=vx[:], in1=vy[:],
                          s0=5.0, s1=2.0)
            ln2 = tmp(F32)
            A.activation(ln2[:], n2[:], AF.Ln, bias=1e-12)
            rn = tmp(F32)
            A.activation(rn[:], ln2[:], AF.Exp, scale=-0.5)
            cb = T_("cb")
            V._custom_dve(OP_SMUL_CLAMP, out=cb[:], in0=vx[:], in1=rn[:],
                          s0=5.0, s1=-1.0, imm2=1.0)
            sb = T_("sb")
            V._custom_dve(OP_SMUL_CLAMP, out=sb[:], in0=vy[:], in1=rn[:],
                          s0=2.0, s1=-1.0, imm2=1.0)
            sh2 = tmp(F32)
            V.scalar_tensor_tensor(sh2[:], cross[:], 10.0, rn[:], ALU.mult,
                                   ALU.mult)
            dHps = T_("dhps")
            V._custom_dve(OP_CLAMP_MUL, out=dHps[:], in0=sh2[:], in1=g12[:],
                          s0=200.0, s1=-1.0, imm2=1.0)

            # ---- T = P(cb) + sb*Q(cb) ----
            u = tmp()
            A.activation(u[:], cb[:], AF.Square)
            cs = tmp()
            V.tensor_mul(cs[:], cb[:], sb[:])
            Pe = tmp()
            V._custom_dve(OP_QUAD, out=Pe[:], in0=u[:], s0=TA4, s1=TA2,
                          imm2=TA0)
            Po = tmp()
            V._custom_dve(OP_AFF_MUL, out=Po[:], in0=u[:], in1=cb[:],
                          s0=TA3, s1=TA1)
            Qe = tmp()
            V._custom_dve(OP_AFF_MUL, out=Qe[:], in0=u[:], in1=sb[:],
                          s0=TQ2, s1=TQ0)
            Qo = tmp()
            V._custom_dve(OP_AFF_MUL, out=Qo[:], in0=u[:], in1=cs[:],
                          s0=TQ3, s1=TQ1)
            t10 = tmp()
            V.tensor_add(t10[:], Pe[:], Po[:])
            t11 = tmp()
            V.tensor_add(t11[:], Qe[:], Qo[:])
            Tt = tmp()
            V.tensor_add(Tt[:], t10[:], t11[:])
            V._custom_dve(OP_MULMUL_ADD, out=S(SCH, 1), in0=Scp[:],
                          in1=Tt[:], s0=0.75, s1=1.0)
            RR = SCH
            V.reciprocal_approx_fast(RR[:], SCH[:])
            tC = T_("tc")
            V.tensor_mul(tC[:], dCp[:], S(RR, 0))
            tH = T_("th")
            V.tensor_mul(tH[:], dHps[:], S(RR, 1))

            # ---- RT gaussian + sin poly ----
            c275 = tmp()
            V._custom_dve(OP_LINCOMB, out=c275[:], in0=cb[:], in1=sb[:],
                          s0=C275, s1=S275, imm2=0.0)
            eg = tmp()
            A.activation(eg[:], c275[:], AF.Exp, scale=10.5, bias=-10.5)
            wg = tmp()
            A.activation(wg[:], eg[:], AF.Square)
            P1 = tmp()
            V.tensor_scalar(P1[:], wg[:], K5, K3, ALU.mult, ALU.add)
            s2d = tmp()
            V._custom_dve(OP_SIN_POLY, out=s2d[:], in0=eg[:], in1=P1[:],
                          s0=K1)
            w1 = tmp()
            V.tensor_mul(w1[:], s2d[:], rsqC[:])

            # ---- F assembly + dE ----
            q1 = tmp()
            V._custom_dve(OP_SUMSQ, out=q1[:], in0=tC[:], in1=tH[:],
                          s0=100.0, s1=1.0)
            q2 = tmp()
            V.scalar_tensor_tensor(q2[:], tC[:], -200.0, tH[:], ALU.mult,
                                   ALU.mult)
            q3 = tmp()
            V.tensor_mul(q3[:], q2[:], w1[:])
            F1 = tmp()
            V.tensor_add(F1[:], q1[:], q3[:])
            Ff = tmp()
            V._custom_dve(OP_ADD_RELU, out=Ff[:], in0=F1[:], in1=zL[:])
            lF = tmp()
            A.activation(lF[:], Ff[:], AF.Ln, bias=1e-30)
            dE = tmp()
            A.activation(dE[:], lF[:], AF.Exp, scale=0.5,
                         accum_out=acc[:, k:k + 1])

            if debug_dump and k == 0:
                for nm, t in [("f", WF), ("al", AL), ("be", BE),
                              ("zl", zL), ("ccp", CCp), ("rsqg", rsqG),
                              ("ap", AP), ("cps", CPS), ("dcp", dCp),
                              ("scp", Scp), ("rsqc", rsqC), ("cross", cross),
                              ("cb", cb), ("sb", sb), ("dhps", dHps),
                              ("tt", Tt), ("sch", SCH), ("tc", tC),
                              ("th", tH), ("eg", eg), ("s2d", s2d),
                              ("ff", Ff), ("de", dE)]:
                    w = t.shape[-1]
                    dd = nc.dram_tensor(f"dbg_{nm}", [P, w], t.dtype,
                                        kind="ExternalOutput").ap()
                    nc.sync.dma_start(dd[:], t[:])

        # ---- software-pipelined driver (2-chunk lookahead on gamma) ----
        stA = {0: stage_A(0)}
        stB = {0: stage_B(stA[0])}
        stA[1] = stage_A(1)
        for k in range(NCHUNK):
            stC = stage_C(stB.pop(k))
            if k + 1 < NCHUNK:
                stB[k + 1] = stage_B(stA.pop(k + 1))
            if k + 2 < NCHUNK:
                stA[k + 2] = stage_A(k + 2)
            stage_D(stC, k)

        # final: reduce acc cols -> [P,1], DMA out
        accsum = pool.tile([P, 1], F32, tag="accsum", name="accsum")
        V.tensor_reduce(accsum[:], acc[:], mybir.AxisListType.X, ALU.add)
        nc.sync.dma_start(out_d[:], accsum[:])

    nc.compile()
    return nc


def _get_nc():
    if "nc" not in _NC_CACHE:
        _NC_CACHE["nc"] = build_nc()
    return _NC_CACHE["nc"]


def kernel(x: np.ndarray, y: np.ndarray) -> np.ndarray:
    assert x.shape == (32, 3, 512, 512) and y.shape == (32, 3, 512, 512)
    nc = _get_nc()
    shp = (IMGS_PER_CORE, 3, ROWS_PER_IMG, NCHUNK, FCH)
    xs = np.ascontiguousarray(x, dtype=np.float32)
    ys = np.ascontiguousarray(y, dtype=np.float32)
    in_maps = []
    for c in range(NCORE):
        xi = xs[c * IMGS_PER_CORE:(c + 1) * IMGS_PER_CORE].reshape(shp)
        yi = ys[c * IMGS_PER_CORE:(c + 1) * IMGS_PER_CORE].reshape(shp)
        in_maps.append({"x": xi, "y": yi})
    trace = bool(int(os.environ.get("COLOR_TRACE", "0")))
    res = run_bass_kernel_spmd(nc, in_maps, core_ids=list(range(NCORE)),
                               trace=trace)
    _NC_CACHE["last_results"] = res
    total = np.float64(0.0)
    for c in range(NCORE):
        total += np.float64(res.results[c]["out"].sum())
    npix = 32 * 512 * 512
    return np.float32(total / npix / 100.0)


# revision 22
# speedup vs baseline: 1.2400x; 1.2400x over previous
"""CIEDE2000 ColorLoss kernel for Trainium2, 8 NeuronCores, data-parallel.

Full inputs x, y: [32, 3, 512, 512] f32 NCHW in [0, 1].
Output: scalar f32 = mean(ciede2000(rgb2lab(x), rgb2lab(y))) / 100.

Sharding: batch dim split 4 images per core (8 cores). Each core computes a
per-partition sum of deltaE over its 4*512*512 pixels; host combines.

Design (v2):
  - Scalar (ACT) engine does ONLY Ln/Exp -> single table set, zero
    ACT_TABLE_LOAD switches after warmup.
  - Zero GpSimd compute (it contends with the DVE on the shared SBUF port).
  - Hue handled algebraically: unit bisector (cos hbar, sin hbar) via
    vector addition u1*C2' + u2*C1'; T weighting as parity-split
    polynomial P(c) + s*Q(c); dtheta gaussian approximated as
    exp(10.5*(cos(hbar-275deg)-1)); dHp = 2000*cross*sqrt(C1'C2')/|v|
    (exact identity, no trig, sign included).
  - Fused custom DVE ops (selects, sum-of-squares, lincombs, polys) with
    immediate constants; bf16 planes for 2x stock-DVE throughput.
  - Math error vs reference ~5e-4 (numpy-simulated, bf16 rounding incl).
"""
import os
import sys

sys.path.insert(0, "/opt/trn_rl_repo")

import numpy as np
import concourse.bacc as bacc
import concourse.tile as tile
import concourse.mybir as mybir
import concourse.dve_ops as D
from concourse.dve_spec import (
    Spec, Src0, Src1, C0, C1, C2, relu, sq, select, maxx, minn,
    lower as dve_lower, _has_src1,
)
from concourse.dve_uop import DveOpSpec
from concourse.bass_utils import run_bass_kernel_spmd
from contextlib import ExitStack

F32 = mybir.dt.float32
BF16 = mybir.dt.bfloat16
AF = mybir.ActivationFunctionType
ALU = mybir.AluOpType

P = 128          # partitions
FCH = 1024       # chunk free dim
NCHUNK = 8       # chunks per core: P*FCH*NCHUNK = 1048576 px = 4 imgs
NCORE = 8
IMGS_PER_CORE = 4
ROWS_PER_IMG = 32  # partitions per image: 262144 / 8192

# ---- constants -------------------------------------------------------------
PI = float(np.pi)
LNP = float(np.log(1.0 / 128.0))     # ln((25/50)^7)
B_LIN = float(0.055 / 1.055)
K1 = float(PI / 3)
K3 = float(-(PI / 3) ** 3 / 6.0)
K5 = float((PI / 3) ** 5 / 120.0)

_M = np.array([[0.412453, 0.357580, 0.180423],
               [0.212671, 0.715160, 0.072169],
               [0.019334, 0.119193, 0.950227]], dtype=np.float64)
_W = np.array([0.95047, 1.0, 1.08883], dtype=np.float64)
MW = (_M / _W[:, None]).astype(np.float32)  # [3,3]

_D2R = PI / 180.0


def _cd(d):
    return float(np.cos(d * _D2R))


def _sd(d):
    return float(np.sin(d * _D2R))


# T(h) = P(cos h) + sin(h) * Q(cos h), parity-split Chebyshev coefficients
TA0 = 1.0 - 0.24 - 0.20 * _cd(63)
TA1 = -0.17 * _cd(30) - 0.96 * _cd(6)
TA2 = 0.48 + 1.6 * _cd(63)
TA3 = 1.28 * _cd(6)
TA4 = -1.6 * _cd(63)
TQ0 = -0.17 * _sd(30) + 0.32 * _sd(6)
TQ1 = 0.80 * _sd(63)
TQ2 = -1.28 * _sd(6)
TQ3 = -1.60 * _sd(63)
C275 = _cd(275)
S275 = _sd(275)

_BIASES = [0.0, B_LIN, LNP, 1.0, 20.0, 1e-12, -10.5, 1e-30]

_NC_CACHE = {}


# ---- custom DVE ops --------------------------------------------------------
def _register_op(name, spec, subdim=False):
    if name in D._SUB_OPCODE_FOR_NAME:
        return next(o for o in D.OPS if o.name == name)
    row = 1 + len(D.OPS)
    assert row < 0x20, "custom DVE opcode rows exhausted"
    D._SUB_OPCODE_FOR_NAME[name] = row
    shas = {}
    for ver in ("v3",):
        s = DveOpSpec(name=name, opcode=row, uops=dve_lower(spec, ver=ver),
                      rd1_en=_has_src1(spec))
        shas[ver] = s.sha(ver)
    op = D.DveOp(name, spec, subdim, shas)
    D.OPS.append(op)
    D.CUSTOM_DVE_SPECS[name] = spec
    return op


# select(x > c0, a, x*c1 + c2) : sRGB + cbrt branch
OP_SEL_GT = _register_op("ANT_SEL_GT", Spec(
    body=select(Src0 > C0, Src1, Src0 * C1 + C2)))
# sq(a*c0) + sq(b*c1) : chroma^2, |v|^2, dE^2 partials
OP_SUMSQ = _register_op("ANT_SUMSQ", Spec(
    body=sq(Src0 * C0) + sq(Src1 * C1)))
# a*c0 + b*c1 + c2
OP_LINCOMB = _register_op("ANT_LINCOMB", Spec(
    body=Src0 * C0 + Src1 * C1 + C2))
# sq((a+b)*c0 + c1) : L50^2
OP_ADD_AFF_SQ = _register_op("ANT_ADD_AFF_SQ", Spec(
    body=sq((Src0 + Src1) * C0 + C1)))
# (a*c0)*b + c1 : SL, SH
OP_MULMUL_ADD = _register_op("ANT_MULMUL_ADD", Spec(
    body=(Src0 * C0) * Src1 + C1))
# sq(a*c0)*sq(b) : zL
OP_SQSQ_MUL = _register_op("ANT_SQSQ_MUL", Spec(
    body=sq(Src0 * C0) * sq(Src1)))
# (a*c0 + c1)*b : (1+G)*alpha, T parity pieces
OP_AFF_MUL = _register_op("ANT_AFF_MUL", Spec(
    body=(Src0 * C0 + C1) * Src1))
# sq(a)*c0 + a*c1 + c2 : quad poly
OP_QUAD = _register_op("ANT_QUAD", Spec(
    body=sq(Src0) * C0 + Src0 * C1 + C2))
# (sq(a)*b + c0)*a : sin odd-poly tail
OP_SIN_POLY = _register_op("ANT_SIN_POLY", Spec(
    body=(sq(Src0) * Src1 + C0) * Src0))
# relu(a + b) : final F clamp
OP_ADD_RELU = _register_op("ANT_ADD_RELU", Spec(
    body=relu(Src0 + Src1)))
# clamp(a*c0*b, c1, c2) : unit-vector components (degenerate-hue guard)
OP_SMUL_CLAMP = _register_op("ANT_SMUL_CLAMP", Spec(
    body=minn(maxx(Src0 * C0 * Src1, C1), C2)))
# clamp(a, c1, c2)*b*c0 : dHp = 200*g12*clamp(sin(dh/2))
OP_CLAMP_MUL = _register_op("ANT_CLAMP_MUL", Spec(
    body=minn(maxx(Src0, C1), C2) * Src1 * C0))
# select(a <= c0, b, a*c1 + c2) : cbrt branch keyed on cbr itself
OP_SEL_LE = _register_op("ANT_SEL_LE", Spec(
    body=select(Src0 <= C0, Src1, Src0 * C1 + C2)))


# Force Ln and Exp to resolve to the combined natural_log_exp set: the
# greedy table-load pass otherwise alternates natural_log <-> exp_and_others
# on every Ln/Exp switch (~2.7us per ACT_TABLE_LOAD, ~20 per chunk).
_ORIG_GAT = None


def _install_lnexp_table_patch():
    global _ORIG_GAT
    if _ORIG_GAT is not None:
        return
    import concourse.hw_specs as hw_specs
    _ORIG_GAT = hw_specs.get_activation_tables

    def _gat(arch):
        t = _ORIG_GAT(arch)
        out = {}
        for name, fns in t.items():
            if name != "natural_log_exp_and_others":
                fns = {f for f in fns if f not in (AF.Ln, AF.Exp)}
            out[name] = fns
        return out

    hw_specs.get_activation_tables = _gat
    bacc.get_activation_tables = _gat


def _reg_consts(nc, values):
    for v in values:
        v = float(v)
        if (F32, v) not in nc.const_aps.aps:
            t = nc.alloc_sbuf_tensor(f"constf32_{repr(v)}", [128, 1], F32)
            nc.gpsimd.memset(t.ap(), v)
            nc.const_aps.aps[(F32, v)] = t.ap()
    nc.all_engine_barrier()


def build_nc(debug_dump=False):
    _install_lnexp_table_patch()
    nc = bacc.Bacc("TRN2", target_bir_lowering=False, debug=False)
    _reg_consts(nc, _BIASES)
    A = nc.scalar
    V = nc.vector

    # inputs viewed as [img, ch, row, chunk, col]
    shp = [IMGS_PER_CORE, 3, ROWS_PER_IMG, NCHUNK, FCH]
    x_d = nc.dram_tensor("x", shp, F32, kind="ExternalInput").ap()
    y_d = nc.dram_tensor("y", shp, F32, kind="ExternalInput").ap()
    out_d = nc.dram_tensor("out", [P, 1], F32, kind="ExternalOutput").ap()

    dbg_tiles = {}

    def dbg(name, t):
        if debug_dump:
            dbg_tiles[name] = (t, t.shape[1], t.dtype)

    W = FCH          # single-plane width
    W2 = 2 * FCH     # pair width
    W6 = 6 * FCH

    with tile.TileContext(nc) as tc, ExitStack() as ctx:
        inpool = ctx.enter_context(tc.tile_pool(name="in", bufs=1))
        pool = ctx.enter_context(tc.tile_pool(name="main", bufs=1))

        NTMP = 8
        NTMPF = 5
        tmp_i = [0, 0]

        def T_(tag, w=W, dt=BF16):
            return pool.tile([P, w], dt, tag=tag, name=tag)

        def tmp(dt=BF16):
            if dt is F32:
                tag = f"ftmp{tmp_i[1] % NTMPF}"
                tmp_i[1] += 1
            else:
                tag = f"tmp{tmp_i[0] % NTMP}"
                tmp_i[0] += 1
            return pool.tile([P, W], dt, tag=tag, name=tag)

        acc = pool.tile([P, NCHUNK], F32, tag="acc", name="acc")

        def S(t, i, n=1):
            """Free-dim slice covering planes [i, i+n) of a multi-plane tile."""
            return t[:, i * FCH:(i + n) * FCH]

        def stage_A(k):
            """DMA chunk k + srgb gamma (Act-heavy)."""
            IN = inpool.tile([P, W6], F32, tag="in6", name="in6")
            for c in range(3):
                for img, src in ((0, x_d), (1, y_d)):
                    pl = 2 * c + img
                    for im in range(IMGS_PER_CORE):
                        nc.sync.dma_start(
                            IN[im * ROWS_PER_IMG:(im + 1) * ROWS_PER_IMG,
                               pl * FCH:(pl + 1) * FCH],
                            src[im, c, :, k, :],
                        )
            W1 = T_("w1", W6)   # LNU -> GAM -> LIN
            A.activation(W1[:], IN[:], AF.Ln, scale=float(1 / 1.055),
                         bias=B_LIN)
            A.activation(W1[:], W1[:], AF.Exp, scale=2.4)
            INB = T_("inb", W6)
            A.activation(INB[:], IN[:], AF.Copy, scale=float(1 / 12.92))
            return INB, W1

        def stage_B(st):
            """lin max-blend + xyz (V-heavy)."""
            INB, W1 = st
            V.tensor_tensor(W1[:], INB[:], W1[:], ALU.max)
            XYZ = T_("xyz", W6)
            for kk in range(3):
                mk = MW[kk]
                t0 = pool.tile([P, W2], BF16, tag="xta", name="xta")
                t1 = pool.tile([P, W2], BF16, tag="xtb", name="xtb")
                V.tensor_scalar(t0[:], S(W1, 0, 2), float(mk[0]), None,
                                ALU.mult)
                V.tensor_scalar(t1[:], S(W1, 2, 2), float(mk[1]), None,
                                ALU.mult)
                V.tensor_add(t0[:], t0[:], t1[:])
                V.tensor_scalar(t1[:], S(W1, 4, 2), float(mk[2]), None,
                                ALU.mult)
                V.tensor_add(S(XYZ, 2 * kk, 2), t0[:], t1[:])
            return (XYZ,)

        def stage_C(st):
            """cbrt Ln/Exp + line plane (Act-heavy)."""
            (XYZ,) = st
            LINE = T_("line", W6)
            A.activation(LINE[:], XYZ[:], AF.Copy, scale=7.787,
                         bias=0.13793103)
            V.tensor_scalar(LINE[:], LINE[:], 0.20689655, None, ALU.min)
            WF = T_("wf", W6)   # LNX -> CBR -> F
            A.activation(WF[:], XYZ[:], AF.Ln)
            A.activation(WF[:], WF[:], AF.Exp, scale=float(1 / 3))
            return WF, LINE

        def stage_D(st, k):
            """cbrt select + full back-end for chunk k."""
            WF, LINE = st
            # f = max(cbr, min(line, cbrt(eps))) -- exact branch blend
            V.tensor_tensor(WF[:], WF[:], LINE[:], ALU.max)

            AL = T_("al", W2)
            BE = T_("be", W2)
            V.tensor_sub(AL[:], S(WF, 0, 2), S(WF, 2, 2))
            V.tensor_sub(BE[:], S(WF, 2, 2), S(WF, 4, 2))

            # ---- L chain ----
            L50 = T_("l50")
            V._custom_dve(OP_ADD_AFF_SQ, out=L50[:], in0=S(WF, 2),
                          in1=S(WF, 3), s0=58.0, s1=-66.0)
            lld = tmp()
            A.activation(lld[:], L50[:], AF.Ln, bias=20.0)
            rLd = tmp()
            A.activation(rLd[:], lld[:], AF.Exp, scale=-0.5)
            SLf = T_("slf", W, F32)
            V._custom_dve(OP_MULMUL_ADD, out=SLf[:], in0=L50[:], in1=rLd[:],
                          s0=0.015, s1=1.0)
            rL = T_("rl", W, F32)
            V.reciprocal_approx_fast(rL[:], SLf[:])
            dfy = T_("dfy")
            V.tensor_sub(dfy[:], S(WF, 3), S(WF, 2))
            zL = T_("zl")
            V._custom_dve(OP_SQSQ_MUL, out=zL[:], in0=dfy[:], in1=rL[:],
                          s0=116.0)

            # ---- chroma + G ----
            C2p = T_("c2p", W2)
            V._custom_dve(OP_SUMSQ, out=C2p[:], in0=AL[:], in1=BE[:],
                          s0=5.0, s1=2.0)
            A.activation(C2p[:], C2p[:], AF.Ln)
            CCp = C2p
            A.activation(CCp[:], CCp[:], AF.Exp, scale=0.5)
            Sc = tmp()
            V.tensor_add(Sc[:], S(CCp, 0), S(CCp, 1))
            lcS = tmp()
            A.activation(lcS[:], Sc[:], AF.Ln)
            e1 = tmp()
            A.activation(e1[:], lcS[:], AF.Exp, scale=-7.0, bias=LNP)
            l1g = tmp()
            A.activation(l1g[:], e1[:], AF.Ln, bias=1.0)
            rsqG = tmp()
            A.activation(rsqG[:], l1g[:], AF.Exp, scale=-0.5)
            AP = T_("ap", W2)
            V._custom_dve(OP_AFF_MUL, out=S(AP, 0), in0=rsqG[:],
                          in1=S(AL, 0), s0=-0.5, s1=1.5)
            V._custom_dve(OP_AFF_MUL, out=S(AP, 1), in0=rsqG[:],
                          in1=S(AL, 1), s0=-0.5, s1=1.5)

            CP2 = T_("cp2", W2)
            V._custom_dve(OP_SUMSQ, out=CP2[:], in0=AP[:], in1=BE[:],
                          s0=5.0, s1=2.0)
            LP = CP2
            A.activation(LP[:], CP2[:], AF.Ln)
            # CPS = [Cpy | Cpx]  (swapped -> pair products via one TT mult)
            CPS = T_("cps", W2)
            A.activation(S(CPS, 0), S(LP, 1), AF.Exp, scale=0.5)
            A.activation(S(CPS, 1), S(LP, 0), AF.Exp, scale=0.5)
            dCp = T_("dcp")
            V.tensor_sub(dCp[:], S(CPS, 0), S(CPS, 1))
            Scp = T_("scp")
            V.tensor_add(Scp[:], S(CPS, 0), S(CPS, 1))
            # SCH = [SC | SH] fp32 for the paired reciprocal
            SCH = T_("sch", W2, F32)
            V.tensor_scalar(S(SCH, 0), Scp[:], 2.25, 1.0, ALU.mult, ALU.add)
            lcp = tmp()
            A.activation(lcp[:], Scp[:], AF.Ln)
            e2 = tmp()
            A.activation(e2[:], lcp[:], AF.Exp, scale=-7.0, bias=LNP)
            l2g = tmp()
            A.activation(l2g[:], e2[:], AF.Ln, bias=1.0)
            rsqC = T_("rsqc")
            A.activation(rsqC[:], l2g[:], AF.Exp, scale=-0.5)

            # ---- hue: cross, sqrt(C1C2), bisector (cb, sb) ----
            # fp32 chain: bf16*bf16 products are exact in fp32, keeping the
            # dHp identity exact; clamps guard the near-antipodal tail.
            m1 = tmp(F32)
            V.tensor_mul(m1[:], S(AP, 0), S(BE, 1))
            m2 = tmp(F32)
            V.tensor_mul(m2[:], S(AP, 1), S(BE, 0))
            cross = T_("cross", W, F32)
            V.tensor_sub(cross[:], m1[:], m2[:])
            lsum = tmp()
            V.tensor_add(lsum[:], S(LP, 0), S(LP, 1))
            g12 = tmp()
            A.activation(g12[:], lsum[:], AF.Exp, scale=0.25)
            prA = pool.tile([P, W2], BF16, tag="pra", name="pra")
            V.tensor_mul(prA[:], AP[:], CPS[:])
            vx = tmp(F32)
            V.tensor_add(vx[:], S(prA, 0), S(prA, 1))
            prB = pool.tile([P, W2], BF16, tag="pra", name="prb")
            V.tensor_mul(prB[:], BE[:], CPS[:])
            vy = tmp(F32)
            V.tensor_add(vy[:], S(prB, 0), S(prB, 1))
            n2 = tmp(F32)
            V._custom_dve(OP_SUMSQ, out=n2[:], in0=vx[:], in1=vy[:],
                          s0=5.0, s1=2.0)
            ln2 = tmp(F32)
            A.activation(ln2[:], n2[:], AF.Ln, bias=1e-12)
            rn = tmp(F32)
            A.activation(rn[:], ln2[:], AF.Exp, scale=-0.5)
            cb = T_("cb")
            V._custom_dve(OP_SMUL_CLAMP, out=cb[:], in0=vx[:], in1=rn[:],
                          s0=5.0, s1=-1.0, imm2=1.0)
            sb = T_("sb")
            V._custom_dve(OP_SMUL_CLAMP, out=sb[:], in0=vy[:], in1=rn[:],
                          s0=2.0, s1=-1.0, imm2=1.0)
            sh2 = tmp(F32)
            V.scalar_tensor_tensor(sh2[:], cross[:], 10.0, rn[:], ALU.mult,
                                   ALU.mult)
            dHps = T_("dhps")
            V._custom_dve(OP_CLAMP_MUL, out=dHps[:], in0=sh2[:], in1=g12[:],
                          s0=200.0, s1=-1.0, imm2=1.0)

            # ---- T = P(cb) + sb*Q(cb) ----
            u = tmp()
            A.activation(u[:], cb[:], AF.Square)
            cs = tmp()
            V.tensor_mul(cs[:], cb[:], sb[:])
            Pe = tmp()
            V._custom_dve(OP_QUAD, out=Pe[:], in0=u[:], s0=TA4, s1=TA2,
                          imm2=TA0)
            Po = tmp()
            V._custom_dve(OP_AFF_MUL, out=Po[:], in0=u[:], in1=cb[:],
                          s0=TA3, s1=TA1)
            Qe = tmp()
            V._custom_dve(OP_AFF_MUL, out=Qe[:], in0=u[:], in1=sb[:],
                          s0=TQ2, s1=TQ0)
            Qo = tmp()
            V._custom_dve(OP_AFF_MUL, out=Qo[:], in0=u[:], in1=cs[:],
                          s0=TQ3, s1=TQ1)
            t10 = tmp()
            V.tensor_add(t10[:], Pe[:], Po[:])
            t11 = tmp()
            V.tensor_add(t11[:], Qe[:], Qo[:])
            Tt = tmp()
            V.tensor_add(Tt[:], t10[:], t11[:])
            V._custom_dve(OP_MULMUL_ADD, out=S(SCH, 1), in0=Scp[:],
                          in1=Tt[:], s0=0.75, s1=1.0)
            RR = SCH
            V.reciprocal_approx_fast(RR[:], SCH[:])
            tC = T_("tc")
            V.tensor_mul(tC[:], dCp[:], S(RR, 0))
            tH = T_("th")
            V.tensor_mul(tH[:], dHps[:], S(RR, 1))

            # ---- RT gaussian + sin poly ----
            c275 = tmp()
            V._custom_dve(OP_LINCOMB, out=c275[:], in0=cb[:], in1=sb[:],
                          s0=C275, s1=S275, imm2=0.0)
            eg = tmp()
            A.activation(eg[:], c275[:], AF.Exp, scale=10.5, bias=-10.5)
            P1 = tmp()
            V._custom_dve(OP_QUAD, out=P1[:], in0=eg[:], s0=K5, s1=0.0,
                          imm2=K3)
            s2d = tmp()
            V._custom_dve(OP_SIN_POLY, out=s2d[:], in0=eg[:], in1=P1[:],
                          s0=K1)
            w1 = tmp()
            V.tensor_mul(w1[:], s2d[:], rsqC[:])

            # ---- F assembly + dE ----
            q1 = tmp()
            V._custom_dve(OP_SUMSQ, out=q1[:], in0=tC[:], in1=tH[:],
                          s0=100.0, s1=1.0)
            q2 = tmp()
            V.scalar_tensor_tensor(q2[:], tC[:], -200.0, tH[:], ALU.mult,
                                   ALU.mult)
            q3 = tmp()
            V.tensor_mul(q3[:], q2[:], w1[:])
            F1 = tmp()
            V.tensor_add(F1[:], q1[:], q3[:])
            Ff = tmp()
            V._custom_dve(OP_ADD_RELU, out=Ff[:], in0=F1[:], in1=zL[:])
            lF = tmp()
            A.activation(lF[:], Ff[:], AF.Ln, bias=1e-30)
            dE = tmp()
            A.activation(dE[:], lF[:], AF.Exp, scale=0.5,
                         accum_out=acc[:, k:k + 1])

            if debug_dump and k == 0:
                for nm, t in [("f", WF), ("al", AL), ("be", BE),
                              ("zl", zL), ("ccp", CCp), ("rsqg", rsqG),
                              ("ap", AP), ("cps", CPS), ("dcp", dCp),
                              ("scp", Scp), ("rsqc", rsqC), ("cross", cross),
                              ("cb", cb), ("sb", sb), ("dhps", dHps),
                              ("tt", Tt), ("sch", SCH), ("tc", tC),
                              ("th", tH), ("eg", eg), ("s2d", s2d),
                              ("ff", Ff), ("de", dE)]:
                    w = t.shape[-1]
                    dd = nc.dram_tensor(f"dbg_{nm}", [P, w], t.dtype,
                                        kind="ExternalOutput").ap()
                    nc.sync.dma_start(dd[:], t[:])

        # ---- software-pipelined driver (2-chunk lookahead on gamma) ----
        stA = {0: stage_A(0)}
        stB = {0: stage_B(stA[0])}
        stA[1] = stage_A(1)
        for k in range(NCHUNK):
            stC = stage_C(stB.pop(k))
            if k + 1 < NCHUNK:
                stB[k + 1] = stage_B(stA.pop(k + 1))
            if k + 2 < NCHUNK:
                stA[k + 2] = stage_A(k + 2)
            stage_D(stC, k)

        # final: reduce acc cols -> [P,1], DMA out
        accsum = pool.tile([P, 1], F32, tag="accsum", name="accsum")
        V.tensor_reduce(accsum[:], acc[:], mybir.AxisListType.X, ALU.add)
        nc.sync.dma_start(out_d[:], accsum[:])

    nc.compile()
    return nc


def _get_nc():
    if "nc" not in _NC_CACHE:
        _NC_CACHE["nc"] = build_nc()
    return _NC_CACHE["nc"]


def kernel(x: np.ndarray, y: np.ndarray) -> np.ndarray:
    assert x.shape == (32, 3, 512, 512) and y.shape == (32, 3, 512, 512)
    nc = _get_nc()
    shp = (IMGS_PER_CORE, 3, ROWS_PER_IMG, NCHUNK, FCH)
    xs = np.ascontiguousarray(x, dtype=np.float32)
    ys = np.ascontiguousarray(y, dtype=np.float32)
    in_maps = []
    for c in range(NCORE):
        xi = xs[c * IMGS_PER_CORE:(c + 1) * IMGS_PER_CORE].reshape(shp)
        yi = ys[c * IMGS_PER_CORE:(c + 1) * IMGS_PER_CORE].reshape(shp)
        in_maps.append({"x": xi, "y": yi})
    trace = bool(int(os.environ.get("COLOR_TRACE", "0")))
    res = run_bass_kernel_spmd(nc, in_maps, core_ids=list(range(NCORE)),
                               trace=trace)
    _NC_CACHE["last_results"] = res
    total = np.float64(0.0)
    for c in range(NCORE):
        total += np.float64(res.results[c]["out"].sum())
    npix = 32 * 512 * 512
    return np.float32(total / npix / 100.0)
